# revision 24
# baseline (speedup 1.0000x reference)
"""Int4 quantized linear (y = x @ dequant(packed, scale).T + bias) on 8 Trainium2 cores.

Sharding: column-parallel on out_features (11008 = 8 x 1376). Each core gets the
full activation x and a 1376-row shard of packed/scale/bias, computes its y
shard [8192, 1376]; host concatenates shards along the feature axis.

Hybrid-precision device kernel per core (n8s[mi] = fp8 DoubleRow k-pairs for
512-token m-tile mi; mixing 10/9 rides the 2e-2 rel-err gate at minimum cost):
  1. Dequant: packed int16 tiles -> weight caches holding EXACT int4 values
     (q - 7 in [-7, 8], no scale folded):
       w8  [128p, ka_max ko, out] fp8e4 (ints exact in e4m3, global kos 0..ka_max)
       w16 [128p, kb_max ko, out] fp16  (ints exact in fp16, global kos k16_0..31)
     Overlapping kos (k16_0..ka_max) exist in both caches.
  2. Matmul per 128-token subtile: psum[tok, out] accumulates
       n8 DoubleRow fp8 matmuls (256 contraction elements each:
         lhsT = x8[:, 2j:2j+2, tok128], rhs = w8[:, 2j:2j+2, outN]) +
       (32-2*n8) fp16 matmuls (128 contraction elements each).
     x is pre-permuted/pre-quantized on the host identically to the weight
     nibble order, so the dot product is unchanged. m-tile 0 issues the DR
     matmuls of 8 psum groups first, then the fp16 k-slices k-outer, so the
     PE is never starved while the DVE dequant pipeline fills.
  3. Epilogue: y = psum * scale (DVE) + bias (GpSimd), DMA to DRAM.

The only meaningful quantization error is e4m3(x) on the fp8 fraction
f = mean(n8s)/16 of the contraction: rel_err ~= 0.0265 * sqrt(f).
"""

import numpy as np
import ml_dtypes

P = 128
OUT, IN = 11008, 4096
B, S = 4, 2048
TOK = B * S
NCORES = 8
M_TILE = 512
# fp8 DoubleRow k-pairs (of 16) per 512-token m-tile; f = n8/16 of K in fp8
N8S = [10] * 8 + [9] * 8

_PROGRAM_CACHE = {}


def _splits(total, step):
    return [(s, min(step, total - s)) for s in range(0, total, step)]


def build_program(tok=TOK, in_dim=IN, out_sh=OUT // NCORES, m_tile=M_TILE,
                  n_tile=512, n8s=None, dr_fd=512):
    """Build and compile the per-core Bass program."""
    import concourse.bacc as bacc
    import concourse.mybir as mybir
    import concourse.tile as tile

    dt = mybir.dt
    alu = mybir.AluOpType

    ko_n = in_dim // P          # 32 k-tiles of depth 128
    nh = in_dim // 2 // P       # 16 packed-halves tiles
    m_tiles = _splits(tok, m_tile)
    if n8s is None:
        n8s = list(N8S)
    assert len(n8s) == len(m_tiles)
    ka_max = 2 * max(n8s)       # fp8 cache covers global kos [0, ka_max)
    k16_0 = 2 * min(n8s)        # fp16 cache covers global kos [k16_0, ko_n)
    kb_max = ko_n - k16_0
    msub = m_tile // P
    n_tiles = _splits(out_sh, n_tile)

    nc = bacc.Bacc("TRN2", target_bir_lowering=False, debug=False,
                   num_devices=NCORES)

    x8_3 = nc.dram_tensor("x8", [P, ka_max, tok], dt.float8e4,
                          kind="ExternalInput").ap()
    x16_3 = nc.dram_tensor("x16", [P, kb_max, tok], dt.float16,
                           kind="ExternalInput").ap()
    pk3 = nc.dram_tensor("pk3", [P, nh, out_sh], dt.int16, kind="ExternalInput").ap()
    scale_bc = nc.dram_tensor("scale_bc", [P, out_sh], dt.float32, kind="ExternalInput").ap()
    bias_bc = nc.dram_tensor("bias_bc", [P, out_sh], dt.float32, kind="ExternalInput").ap()
    y = nc.dram_tensor("y", [tok, out_sh], dt.float32, kind="ExternalOutput").ap()

    with tile.TileContext(nc) as tc:
        with tc.tile_pool(name="const", bufs=1) as cpool, \
             tc.tile_pool(name="wcache", bufs=1) as wpool, \
             tc.tile_pool(name="pkpool", bufs=4) as pkpool, \
             tc.tile_pool(name="deq", bufs=3) as dqpool, \
             tc.tile_pool(name="xin", bufs=2) as xpool, \
             tc.tile_pool(name="yout", bufs=8) as ypool, \
             tc.tile_pool(name="psum", bufs=8, space="PSUM") as pspool:

            # DMA order: pk h0 (tiny, unblocks dequant) -> x m-tile 0 (first
            # matmul operands) -> remaining pk tiles -> scale/bias (needed
            # only by the first epilogue ~50us in).
            xt_pre = {}

            def load_x(mi, m0, mlen):
                ka = 2 * n8s[mi]
                s16 = ka - k16_0
                xt8 = xpool.tile([P, ka_max, m_tile], dt.float8e4, name="xt8")
                nc.sync.dma_start(out=xt8[:, :ka, :mlen],
                                  in_=x8_3[:, :ka, m0:m0 + mlen])
                xt16 = xpool.tile([P, kb_max, m_tile], dt.float16, name="xt16")
                nc.sync.dma_start(out=xt16[:, s16:, :mlen],
                                  in_=x16_3[:, s16:, m0:m0 + mlen])
                return xt8, xt16

            pk_tiles = {}

            def load_pk(h):
                pk = pkpool.tile([P, out_sh], dt.int16, name="pk")
                nc.sync.dma_start(out=pk[:], in_=pk3[:, h, :])
                pk_tiles[h] = pk

            load_pk(0)
            # m-tile 0's x8 in two chunks so the first DR slices land first;
            # its x16 is not needed until ~30us in, so it queues after pk.
            m0, mlen0 = m_tiles[0]
            ka0 = 2 * n8s[0]
            xt8_0 = xpool.tile([P, ka_max, m_tile], dt.float8e4, name="xt8")
            nc.sync.dma_start(out=xt8_0[:, :6, :mlen0],
                              in_=x8_3[:, :6, m0:m0 + mlen0])
            nc.sync.dma_start(out=xt8_0[:, 6:ka0, :mlen0],
                              in_=x8_3[:, 6:ka0, m0:m0 + mlen0])
            for h in range(1, nh):
                load_pk(h)
            xt16_0 = xpool.tile([P, kb_max, m_tile], dt.float16, name="xt16")
            s16_0 = ka0 - k16_0
            nc.sync.dma_start(out=xt16_0[:, s16_0:, :mlen0],
                              in_=x16_3[:, s16_0:, m0:m0 + mlen0])
            xt_pre[0] = (xt8_0, xt16_0)

            # --- dequant: weight caches with EXACT ints (no scale) ---
            # Pass 1 covers out-columns [0, c_crit) = what m-tile 0's eight
            # warm-up psum groups consume; pass 2 (re-DMA'd pk slices) fills
            # the rest off the PE-critical path.
            c_crit = min(2 * n_tile, out_sh)
            w8 = wpool.tile([P, ka_max, out_sh], dt.float8e4, name="w8")
            w16 = wpool.tile([P, kb_max, out_sh], dt.float16, name="w16")

            def dequant(h, pk, c0, c1):
                for lo in range(2):
                    ko = 2 * h + lo
                    q = dqpool.tile([P, out_sh], dt.int16, name="q")
                    if lo == 0:
                        nc.vector.tensor_scalar(
                            q[:, c0:c1], pk[:, c0:c1], 15, None,
                            alu.bitwise_and)
                    else:
                        nc.vector.tensor_scalar(
                            q[:, c0:c1], pk[:, c0:c1], 4, None,
                            alu.logical_shift_right)
                    if ko < ka_max:
                        nc.vector.tensor_scalar(
                            w8[:, ko, c0:c1], q[:, c0:c1], 7, None,
                            alu.subtract)
                    else:
                        nc.vector.tensor_scalar(
                            w16[:, ko - k16_0, c0:c1], q[:, c0:c1], 7, None,
                            alu.subtract)

            for h in range(nh):
                dequant(h, pk_tiles.pop(h), 0, c_crit)
            if c_crit < out_sh:
                # pkpool recycles buffers; re-DMA the residual column slice
                ctail = out_sh - c_crit
                for h in range(nh):
                    pk2_full = pkpool.tile([P, out_sh], dt.int16, name="pk2")
                    pk2 = pk2_full[:, :ctail]
                    nc.sync.dma_start(out=pk2, in_=pk3[:, h, c_crit:out_sh])
                    for lo in range(2):
                        ko = 2 * h + lo
                        q = dqpool.tile([P, out_sh], dt.int16, name="q")
                        if lo == 0:
                            nc.vector.tensor_scalar(
                                q[:, :ctail], pk2, 15, None, alu.bitwise_and)
                        else:
                            nc.vector.tensor_scalar(
                                q[:, :ctail], pk2, 4, None,
                                alu.logical_shift_right)
                        if ko < ka_max:
                            nc.vector.tensor_scalar(
                                w8[:, ko, c_crit:out_sh], q[:, :ctail], 7,
                                None, alu.subtract)
                        else:
                            nc.vector.tensor_scalar(
                                w16[:, ko - k16_0, c_crit:out_sh],
                                q[:, :ctail], 7, None, alu.subtract)
            # kos in [k16_0, ka_max) live in both caches; fill the fp16 copy
            # off the dequant critical path (first needed by m-tile 8)
            for ko in range(k16_0, ka_max):
                nc.vector.tensor_copy(out=w16[:, ko - k16_0, :],
                                      in_=w8[:, ko, :])

            scale_t = cpool.tile([P, out_sh], dt.float32)
            nc.sync.dma_start(out=scale_t[:], in_=scale_bc)
            bias_t = cpool.tile([P, out_sh], dt.float32)
            nc.sync.dma_start(out=bias_t[:], in_=bias_bc)

            def emit_dr(ps_full, xt8, n8, ms, n0, fd, close=False):
                for f0, flen in _splits(fd, dr_fd):
                    for j in range(n8):
                        nc.tensor.matmul(
                            ps_full[:, f0:f0 + flen],
                            lhsT=xt8[:, 2 * j:2 * j + 2, ms * P:(ms + 1) * P],
                            rhs=w8[:, 2 * j:2 * j + 2, n0 + f0:n0 + f0 + flen],
                            start=(j == 0),
                            stop=(close and j == n8 - 1),
                            perf_mode=mybir.MatmulPerfMode.DoubleRow,
                        )

            def emit_fp16_ko(ps_full, xt16, ko, ms, n0, fd, stop):
                nc.tensor.matmul(
                    ps_full[:, :fd],
                    lhsT=xt16[:, ko - k16_0, ms * P:(ms + 1) * P],
                    rhs=w16[:, ko - k16_0, n0:n0 + fd],
                    start=False,
                    stop=stop,
                )

            def emit_epilogue(ps_full, mi, ms, m0, n0, fd):
                yt_full = ypool.tile([P, n_tile], dt.float32, name="yt")
                yt = yt_full[:, :fd]
                if mi == len(m_tiles) - 1:
                    # final m-tile: shortest chain (DVE is idle by then)
                    nc.vector.tensor_mul(
                        out=yt, in0=ps_full[:, :fd],
                        in1=scale_t[:, n0:n0 + fd])
                    nc.vector.tensor_add(
                        out=yt, in0=yt, in1=bias_t[:, n0:n0 + fd])
                else:
                    # ACT drains psum (frees the bank without queuing on
                    # DVE's dequant backlog); scale+bias on idle GpSimd
                    nc.scalar.copy(out=yt, in_=ps_full[:, :fd])
                    nc.gpsimd.tensor_mul(
                        out=yt, in0=yt, in1=scale_t[:, n0:n0 + fd])
                    nc.gpsimd.tensor_add(
                        out=yt, in0=yt, in1=bias_t[:, n0:n0 + fd])
                nc.sync.dma_start(
                    out=y[m0 + ms * P:m0 + (ms + 1) * P, n0:n0 + fd],
                    in_=yt)

            # --- matmul + epilogue ---
            for mi, (m0, mlen) in enumerate(m_tiles):
                n8 = n8s[mi]
                ka = 2 * n8
                xt8, xt16 = xt_pre.pop(mi)
                # prefetch the next m-tile's x ahead of this tile's y writes
                if mi + 1 < len(m_tiles):
                    xt_pre[mi + 1] = load_x(mi + 1, *m_tiles[mi + 1])
                groups = [(n0, fd, ms) for (n0, fd) in n_tiles
                          for ms in range(msub) if ms * P < mlen]
                if mi == 0:
                    # Warm-up schedule: open 8 psum groups on DR-only work
                    # (fp8 cache fills first), then stream their fp16 parts
                    # k-outer so the PE tracks the dequant pipeline.
                    open_g = groups[:8]
                    ps_of = {}
                    for g in open_g:
                        ps_of[g] = pspool.tile([P, n_tile], dt.float32,
                                               name="ps")
                    # j-outer: all groups consume dequant slice j before any
                    # group needs slice j+1 (no head-of-line block on DVE)
                    for j in range(n8):
                        for g in open_g:
                            n0, fd, ms = g
                            nc.tensor.matmul(
                                ps_of[g][:, :fd],
                                lhsT=xt8[:, 2 * j:2 * j + 2,
                                         ms * P:(ms + 1) * P],
                                rhs=w8[:, 2 * j:2 * j + 2, n0:n0 + fd],
                                start=(j == 0),
                                stop=False,
                                perf_mode=mybir.MatmulPerfMode.DoubleRow,
                            )
                    for ko in range(ka, ko_n):
                        for g in open_g:
                            n0, fd, ms = g
                            emit_fp16_ko(ps_of[g], xt16, ko, ms, n0, fd,
                                         stop=(ko == ko_n - 1))
                    for g in open_g:
                        n0, fd, ms = g
                        emit_epilogue(ps_of[g], mi, ms, m0, n0, fd)
                    rest = groups[8:]
                else:
                    rest = groups
                for (n0, fd, ms) in rest:
                    ps_full = pspool.tile([P, n_tile], dt.float32, name="ps")
                    emit_dr(ps_full, xt8, n8, ms, n0, fd)
                    for ko in range(ka, ko_n):
                        emit_fp16_ko(ps_full, xt16, ko, ms, n0, fd,
                                     stop=(ko == ko_n - 1))
                    emit_epilogue(ps_full, mi, ms, m0, n0, fd)

    nc.compile()
    return nc, None


def host_prep_x(x, tok=TOK, in_dim=IN, n8s=None):
    """[tok, in] fp32 -> permuted (x8 [128, ka_max, tok] e4m3,
    x16 [128, kb_max, tok] fp16 covering global kos [k16_0, 32))."""
    nh = in_dim // 2 // P
    if n8s is None:
        n8s = list(N8S)
    ka_max = 2 * max(n8s)
    k16_0 = 2 * min(n8s)
    xf = np.ascontiguousarray(x, dtype=np.float32).reshape(tok, in_dim)
    x4 = xf.reshape(tok, nh, P, 2)                    # [t, h, p, lo]
    x3 = np.ascontiguousarray(x4.transpose(2, 1, 3, 0)).reshape(P, 2 * nh, tok)
    x8 = np.ascontiguousarray(x3[:, :ka_max, :]).astype(ml_dtypes.float8_e4m3)
    x16 = np.ascontiguousarray(x3[:, k16_0:, :]).astype(np.float16)
    return x8, x16


def host_prep_shard(packed, scale, bias, out_sh, in_dim=IN):
    """Per-core shard prep. packed [out_sh, in//2] int32 -> [128, nh, out_sh] int16."""
    nh = in_dim // 2 // P
    pk = np.asarray(packed, dtype=np.int16)           # values 0..255, exact
    pk3 = np.ascontiguousarray(
        pk.T.reshape(nh, P, out_sh).transpose(1, 0, 2))
    sc = np.ascontiguousarray(
        np.broadcast_to(np.asarray(scale, np.float32), (P, out_sh)))
    bi = np.ascontiguousarray(
        np.broadcast_to(np.asarray(bias, np.float32), (P, out_sh)))
    return pk3, sc, bi


def make_in_maps(x, packed, scale, bias, ncores=NCORES):
    out_sh = packed.shape[0] // ncores
    x8, x16 = host_prep_x(x)
    in_maps = []
    for c in range(ncores):
        lo, hi = c * out_sh, (c + 1) * out_sh
        pk3, sc, bi = host_prep_shard(packed[lo:hi], scale[lo:hi], bias[lo:hi], out_sh)
        in_maps.append({"x8": x8, "x16": x16, "pk3": pk3,
                        "scale_bc": sc, "bias_bc": bi})
    return in_maps


def reference_host(x, packed, scale, bias):
    """Numpy reference (for testing only)."""
    q0 = packed & 15
    q1 = (packed >> 4) & 15
    q = np.stack([q0, q1], axis=-1).reshape(packed.shape[0], -1) - 7
    w = q.astype(np.float32) * np.asarray(scale, np.float32)[:, None]
    xf = np.asarray(x, np.float32).reshape(-1, w.shape[1])
    return (xf @ w.T + np.asarray(bias, np.float32)).reshape(
        x.shape[0], x.shape[1], -1)


def quantized_host(x, packed, scale, bias, n8s=None, m_tile=M_TILE):
    """Numpy simulation of exactly what the device computes (testing only)."""
    if n8s is None:
        n8s = list(N8S)
    q0 = packed & 15
    q1 = (packed >> 4) & 15
    q = (np.stack([q0, q1], axis=-1).reshape(packed.shape[0], -1) - 7).astype(
        np.float32)
    xf = np.asarray(x, np.float32).reshape(-1, q.shape[1])
    xq = np.empty_like(xf)
    for mi, n8 in enumerate(n8s):
        s, e = mi * m_tile, (mi + 1) * m_tile
        kc = n8 * 2 * P   # device k order: fp8 part = original cols [0, kc)
        xq[s:e, :kc] = xf[s:e, :kc].astype(ml_dtypes.float8_e4m3).astype(
            np.float32)
        xq[s:e, kc:] = xf[s:e, kc:].astype(np.float16).astype(np.float32)
    ps = xq @ q.T
    yv = ps * np.asarray(scale, np.float32)[None, :] + np.asarray(
        bias, np.float32)[None, :]
    return yv.reshape(x.shape[0], x.shape[1], -1)


def _get_program():
    key = "full"
    if key not in _PROGRAM_CACHE:
        _PROGRAM_CACHE[key] = build_program()
    return _PROGRAM_CACHE[key]


def run_on_hw(inputs, trace=False, trace_kwargs=None):
    """Run the full-size problem on 8 cores. Returns (y_full, BassKernelResults)."""
    from concourse.bass_utils import run_bass_kernel_spmd

    nc, _ = _get_program()
    in_maps = make_in_maps(inputs["x"], inputs["packed"], inputs["scale"],
                           inputs["bias"])
    kw = {}
    if trace:
        kw["trace"] = True
        if trace_kwargs:
            kw["trace_kwargs"] = trace_kwargs
    res = run_bass_kernel_spmd(nc, in_maps, core_ids=list(range(NCORES)), **kw)
    y = np.concatenate([res.results[c]["y"] for c in range(NCORES)], axis=1)
    y = np.ascontiguousarray(y.reshape(B, S, OUT), dtype=np.float32)
    return y, res


def kernel(x, packed, scale, bias):
    y, _ = run_on_hw({"x": x, "packed": packed, "scale": scale, "bias": bias})
    return y


# revision 25
# speedup vs baseline: 1.0040x; 1.0040x over previous
"""Int4 quantized linear (y = x @ dequant(packed, scale).T + bias) on 8 Trainium2 cores.

Sharding: column-parallel on out_features (11008 = 8 x 1376). Each core gets the
full activation x and a 1376-row shard of packed/scale/bias, computes its y
shard [8192, 1376]; host concatenates shards along the feature axis.

Hybrid-precision device kernel per core (n8s[mi] = fp8 DoubleRow k-pairs for
512-token m-tile mi; mixing 10/9 rides the 2e-2 rel-err gate at minimum cost):
  1. Dequant: packed int16 tiles -> weight caches holding EXACT int4 values
     (q - 7 in [-7, 8], no scale folded):
       w8  [128p, ka_max ko, out] fp8e4 (ints exact in e4m3, global kos 0..ka_max)
       w16 [128p, kb_max ko, out] fp16  (ints exact in fp16, global kos k16_0..31)
     Overlapping kos (k16_0..ka_max) exist in both caches.
  2. Matmul per 128-token subtile: psum[tok, out] accumulates
       n8 DoubleRow fp8 matmuls (256 contraction elements each:
         lhsT = x8[:, 2j:2j+2, tok128], rhs = w8[:, 2j:2j+2, outN]) +
       (32-2*n8) fp16 matmuls (128 contraction elements each).
     x is pre-permuted/pre-quantized on the host identically to the weight
     nibble order, so the dot product is unchanged. m-tile 0 issues the DR
     matmuls of 8 psum groups first, then the fp16 k-slices k-outer, so the
     PE is never starved while the DVE dequant pipeline fills.
  3. Epilogue: y = psum * scale (DVE) + bias (GpSimd), DMA to DRAM.

The only meaningful quantization error is e4m3(x) on the fp8 fraction
f = mean(n8s)/16 of the contraction: rel_err ~= 0.0265 * sqrt(f).
"""

import numpy as np
import ml_dtypes

P = 128
OUT, IN = 11008, 4096
B, S = 4, 2048
TOK = B * S
NCORES = 8
M_TILE = 512
# fp8 DoubleRow k-pairs (of 16) per 512-token m-tile; f = n8/16 of K in fp8
N8S = [10] * 8 + [9] * 8

_PROGRAM_CACHE = {}


def _splits(total, step):
    return [(s, min(step, total - s)) for s in range(0, total, step)]


def build_program(tok=TOK, in_dim=IN, out_sh=OUT // NCORES, m_tile=M_TILE,
                  n_tile=512, n8s=None, dr_fd=512):
    """Build and compile the per-core Bass program."""
    import concourse.bacc as bacc
    import concourse.mybir as mybir
    import concourse.tile as tile

    dt = mybir.dt
    alu = mybir.AluOpType

    ko_n = in_dim // P          # 32 k-tiles of depth 128
    nh = in_dim // 2 // P       # 16 packed-halves tiles
    m_tiles = _splits(tok, m_tile)
    if n8s is None:
        n8s = list(N8S)
    assert len(n8s) == len(m_tiles)
    ka_max = 2 * max(n8s)       # fp8 cache covers global kos [0, ka_max)
    k16_0 = 2 * min(n8s)        # fp16 cache covers global kos [k16_0, ko_n)
    kb_max = ko_n - k16_0
    msub = m_tile // P
    n_tiles = _splits(out_sh, n_tile)

    nc = bacc.Bacc("TRN2", target_bir_lowering=False, debug=False,
                   num_devices=NCORES)

    x8_3 = nc.dram_tensor("x8", [P, ka_max, tok], dt.float8e4,
                          kind="ExternalInput").ap()
    x16_3 = nc.dram_tensor("x16", [P, kb_max, tok], dt.float16,
                           kind="ExternalInput").ap()
    pk3 = nc.dram_tensor("pk3", [P, nh, out_sh], dt.int16, kind="ExternalInput").ap()
    scale_bc = nc.dram_tensor("scale_bc", [P, out_sh], dt.float32, kind="ExternalInput").ap()
    bias_bc = nc.dram_tensor("bias_bc", [P, out_sh], dt.float32, kind="ExternalInput").ap()
    y = nc.dram_tensor("y", [tok, out_sh], dt.float32, kind="ExternalOutput").ap()

    with tile.TileContext(nc) as tc:
        with tc.tile_pool(name="const", bufs=1) as cpool, \
             tc.tile_pool(name="wcache", bufs=1) as wpool, \
             tc.tile_pool(name="pkpool", bufs=4) as pkpool, \
             tc.tile_pool(name="deq", bufs=3) as dqpool, \
             tc.tile_pool(name="xin", bufs=2) as xpool, \
             tc.tile_pool(name="yout", bufs=8) as ypool, \
             tc.tile_pool(name="psum", bufs=8, space="PSUM") as pspool:

            # DMA order: pk h0 (tiny, unblocks dequant) -> x m-tile 0 (first
            # matmul operands) -> remaining pk tiles -> scale/bias (needed
            # only by the first epilogue ~50us in).
            xt_pre = {}

            def load_x(mi, m0, mlen):
                ka = 2 * n8s[mi]
                s16 = ka - k16_0
                xt8 = xpool.tile([P, ka_max, m_tile], dt.float8e4, name="xt8")
                nc.sync.dma_start(out=xt8[:, :ka, :mlen],
                                  in_=x8_3[:, :ka, m0:m0 + mlen])
                xt16 = xpool.tile([P, kb_max, m_tile], dt.float16, name="xt16")
                nc.sync.dma_start(out=xt16[:, s16:, :mlen],
                                  in_=x16_3[:, s16:, m0:m0 + mlen])
                return xt8, xt16

            pk_tiles = {}

            def load_pk(h):
                pk = pkpool.tile([P, out_sh], dt.int16, name="pk")
                nc.sync.dma_start(out=pk[:], in_=pk3[:, h, :])
                pk_tiles[h] = pk

            load_pk(0)
            # m-tile 0's x8 in two chunks so the first DR slices land first;
            # its x16 is not needed until ~30us in, so it queues after pk.
            m0, mlen0 = m_tiles[0]
            ka0 = 2 * n8s[0]
            xt8_0 = xpool.tile([P, ka_max, m_tile], dt.float8e4, name="xt8")
            nc.sync.dma_start(out=xt8_0[:, :6, :mlen0],
                              in_=x8_3[:, :6, m0:m0 + mlen0])
            nc.sync.dma_start(out=xt8_0[:, 6:ka0, :mlen0],
                              in_=x8_3[:, 6:ka0, m0:m0 + mlen0])
            for h in range(1, nh):
                load_pk(h)
            xt16_0 = xpool.tile([P, kb_max, m_tile], dt.float16, name="xt16")
            s16_0 = ka0 - k16_0
            nc.sync.dma_start(out=xt16_0[:, s16_0:, :mlen0],
                              in_=x16_3[:, s16_0:, m0:m0 + mlen0])
            xt_pre[0] = (xt8_0, xt16_0)

            # --- dequant: weight caches with EXACT ints (no scale) ---
            # Pass 1 covers out-columns [0, c_crit) = what m-tile 0's eight
            # warm-up psum groups consume; pass 2 (re-DMA'd pk slices) fills
            # the rest off the PE-critical path.
            c_crit = min(2 * n_tile, out_sh)
            w8 = wpool.tile([P, ka_max, out_sh], dt.float8e4, name="w8")
            w16 = wpool.tile([P, kb_max, out_sh], dt.float16, name="w16")

            def dequant(h, pk, c0, c1):
                for lo in range(2):
                    ko = 2 * h + lo
                    q = dqpool.tile([P, out_sh], dt.int16, name="q")
                    if lo == 0:
                        nc.vector.tensor_scalar(
                            q[:, c0:c1], pk[:, c0:c1], 15, None,
                            alu.bitwise_and)
                    else:
                        nc.vector.tensor_scalar(
                            q[:, c0:c1], pk[:, c0:c1], 4, None,
                            alu.logical_shift_right)
                    if ko < ka_max:
                        nc.vector.tensor_scalar(
                            w8[:, ko, c0:c1], q[:, c0:c1], 7, None,
                            alu.subtract)
                    else:
                        nc.vector.tensor_scalar(
                            w16[:, ko - k16_0, c0:c1], q[:, c0:c1], 7, None,
                            alu.subtract)

            for h in range(nh):
                dequant(h, pk_tiles.pop(h), 0, c_crit)
            if c_crit < out_sh:
                # pkpool recycles buffers; re-DMA the residual column slice
                ctail = out_sh - c_crit
                for h in range(nh):
                    pk2_full = pkpool.tile([P, out_sh], dt.int16, name="pk2")
                    pk2 = pk2_full[:, :ctail]
                    nc.sync.dma_start(out=pk2, in_=pk3[:, h, c_crit:out_sh])
                    for lo in range(2):
                        ko = 2 * h + lo
                        q = dqpool.tile([P, out_sh], dt.int16, name="q")
                        if lo == 0:
                            nc.vector.tensor_scalar(
                                q[:, :ctail], pk2, 15, None, alu.bitwise_and)
                        else:
                            nc.vector.tensor_scalar(
                                q[:, :ctail], pk2, 4, None,
                                alu.logical_shift_right)
                        if ko < ka_max:
                            nc.vector.tensor_scalar(
                                w8[:, ko, c_crit:out_sh], q[:, :ctail], 7,
                                None, alu.subtract)
                        else:
                            nc.vector.tensor_scalar(
                                w16[:, ko - k16_0, c_crit:out_sh],
                                q[:, :ctail], 7, None, alu.subtract)
            # kos in [k16_0, ka_max) live in both caches; fill the fp16 copy
            # off the dequant critical path (first needed by m-tile 8)
            for ko in range(k16_0, ka_max):
                nc.vector.tensor_copy(out=w16[:, ko - k16_0, :],
                                      in_=w8[:, ko, :])

            scale_t = cpool.tile([P, out_sh], dt.float32)
            nc.sync.dma_start(out=scale_t[:], in_=scale_bc)
            bias_t = cpool.tile([P, out_sh], dt.float32)
            nc.sync.dma_start(out=bias_t[:], in_=bias_bc)

            def emit_dr(ps_full, xt8, n8, ms, n0, fd, close=False):
                for f0, flen in _splits(fd, dr_fd):
                    for j in range(n8):
                        nc.tensor.matmul(
                            ps_full[:, f0:f0 + flen],
                            lhsT=xt8[:, 2 * j:2 * j + 2, ms * P:(ms + 1) * P],
                            rhs=w8[:, 2 * j:2 * j + 2, n0 + f0:n0 + f0 + flen],
                            start=(j == 0),
                            stop=(close and j == n8 - 1),
                            perf_mode=mybir.MatmulPerfMode.DoubleRow,
                        )

            def emit_fp16_ko(ps_full, xt16, ko, ms, n0, fd, stop):
                nc.tensor.matmul(
                    ps_full[:, :fd],
                    lhsT=xt16[:, ko - k16_0, ms * P:(ms + 1) * P],
                    rhs=w16[:, ko - k16_0, n0:n0 + fd],
                    start=False,
                    stop=stop,
                )

            def emit_epilogue(ps_full, mi, ms, m0, n0, fd):
                yt_full = ypool.tile([P, n_tile], dt.float32, name="yt")
                yt = yt_full[:, :fd]
                if mi == len(m_tiles) - 1:
                    # final m-tile: shortest chain (DVE is idle by then)
                    nc.vector.tensor_mul(
                        out=yt, in0=ps_full[:, :fd],
                        in1=scale_t[:, n0:n0 + fd])
                    nc.vector.tensor_add(
                        out=yt, in0=yt, in1=bias_t[:, n0:n0 + fd])
                else:
                    # ACT drains psum (frees the bank without queuing on
                    # DVE's dequant backlog); scale+bias on idle GpSimd
                    nc.scalar.copy(out=yt, in_=ps_full[:, :fd])
                    nc.gpsimd.tensor_mul(
                        out=yt, in0=yt, in1=scale_t[:, n0:n0 + fd])
                    nc.gpsimd.tensor_add(
                        out=yt, in0=yt, in1=bias_t[:, n0:n0 + fd])
                nc.sync.dma_start(
                    out=y[m0 + ms * P:m0 + (ms + 1) * P, n0:n0 + fd],
                    in_=yt)

            # --- matmul + epilogue ---
            for mi, (m0, mlen) in enumerate(m_tiles):
                n8 = n8s[mi]
                ka = 2 * n8
                if mi in xt_pre:
                    xt8, xt16 = xt_pre.pop(mi)
                else:
                    xt8, xt16 = load_x(mi, m0, mlen)
                groups = [(n0, fd, ms) for (n0, fd) in n_tiles
                          for ms in range(msub) if ms * P < mlen]
                if mi == 0:
                    # Warm-up schedule: open 8 psum groups on DR-only work
                    # (fp8 cache fills first), then stream their fp16 parts
                    # k-outer so the PE tracks the dequant pipeline.
                    open_g = groups[:8]
                    ps_of = {}
                    for g in open_g:
                        ps_of[g] = pspool.tile([P, n_tile], dt.float32,
                                               name="ps")
                    # j-outer: all groups consume dequant slice j before any
                    # group needs slice j+1 (no head-of-line block on DVE)
                    for j in range(n8):
                        for g in open_g:
                            n0, fd, ms = g
                            nc.tensor.matmul(
                                ps_of[g][:, :fd],
                                lhsT=xt8[:, 2 * j:2 * j + 2,
                                         ms * P:(ms + 1) * P],
                                rhs=w8[:, 2 * j:2 * j + 2, n0:n0 + fd],
                                start=(j == 0),
                                stop=False,
                                perf_mode=mybir.MatmulPerfMode.DoubleRow,
                            )
                    for ko in range(ka, ko_n):
                        for g in open_g:
                            n0, fd, ms = g
                            emit_fp16_ko(ps_of[g], xt16, ko, ms, n0, fd,
                                         stop=(ko == ko_n - 1))
                    for g in open_g:
                        n0, fd, ms = g
                        emit_epilogue(ps_of[g], mi, ms, m0, n0, fd)
                    rest = groups[8:]
                else:
                    rest = groups
                for (n0, fd, ms) in rest:
                    ps_full = pspool.tile([P, n_tile], dt.float32, name="ps")
                    emit_dr(ps_full, xt8, n8, ms, n0, fd)
                    for ko in range(ka, ko_n):
                        emit_fp16_ko(ps_full, xt16, ko, ms, n0, fd,
                                     stop=(ko == ko_n - 1))
                    emit_epilogue(ps_full, mi, ms, m0, n0, fd)

    nc.compile()
    return nc, None


def host_prep_x(x, tok=TOK, in_dim=IN, n8s=None):
    """[tok, in] fp32 -> permuted (x8 [128, ka_max, tok] e4m3,
    x16 [128, kb_max, tok] fp16 covering global kos [k16_0, 32))."""
    nh = in_dim // 2 // P
    if n8s is None:
        n8s = list(N8S)
    ka_max = 2 * max(n8s)
    k16_0 = 2 * min(n8s)
    xf = np.ascontiguousarray(x, dtype=np.float32).reshape(tok, in_dim)
    x4 = xf.reshape(tok, nh, P, 2)                    # [t, h, p, lo]
    x3 = np.ascontiguousarray(x4.transpose(2, 1, 3, 0)).reshape(P, 2 * nh, tok)
    x8 = np.ascontiguousarray(x3[:, :ka_max, :]).astype(ml_dtypes.float8_e4m3)
    x16 = np.ascontiguousarray(x3[:, k16_0:, :]).astype(np.float16)
    return x8, x16


def host_prep_shard(packed, scale, bias, out_sh, in_dim=IN):
    """Per-core shard prep. packed [out_sh, in//2] int32 -> [128, nh, out_sh] int16."""
    nh = in_dim // 2 // P
    pk = np.asarray(packed, dtype=np.int16)           # values 0..255, exact
    pk3 = np.ascontiguousarray(
        pk.T.reshape(nh, P, out_sh).transpose(1, 0, 2))
    sc = np.ascontiguousarray(
        np.broadcast_to(np.asarray(scale, np.float32), (P, out_sh)))
    bi = np.ascontiguousarray(
        np.broadcast_to(np.asarray(bias, np.float32), (P, out_sh)))
    return pk3, sc, bi


def make_in_maps(x, packed, scale, bias, ncores=NCORES):
    out_sh = packed.shape[0] // ncores
    x8, x16 = host_prep_x(x)
    in_maps = []
    for c in range(ncores):
        lo, hi = c * out_sh, (c + 1) * out_sh
        pk3, sc, bi = host_prep_shard(packed[lo:hi], scale[lo:hi], bias[lo:hi], out_sh)
        in_maps.append({"x8": x8, "x16": x16, "pk3": pk3,
                        "scale_bc": sc, "bias_bc": bi})
    return in_maps


def reference_host(x, packed, scale, bias):
    """Numpy reference (for testing only)."""
    q0 = packed & 15
    q1 = (packed >> 4) & 15
    q = np.stack([q0, q1], axis=-1).reshape(packed.shape[0], -1) - 7
    w = q.astype(np.float32) * np.asarray(scale, np.float32)[:, None]
    xf = np.asarray(x, np.float32).reshape(-1, w.shape[1])
    return (xf @ w.T + np.asarray(bias, np.float32)).reshape(
        x.shape[0], x.shape[1], -1)


def quantized_host(x, packed, scale, bias, n8s=None, m_tile=M_TILE):
    """Numpy simulation of exactly what the device computes (testing only)."""
    if n8s is None:
        n8s = list(N8S)
    q0 = packed & 15
    q1 = (packed >> 4) & 15
    q = (np.stack([q0, q1], axis=-1).reshape(packed.shape[0], -1) - 7).astype(
        np.float32)
    xf = np.asarray(x, np.float32).reshape(-1, q.shape[1])
    xq = np.empty_like(xf)
    for mi, n8 in enumerate(n8s):
        s, e = mi * m_tile, (mi + 1) * m_tile
        kc = n8 * 2 * P   # device k order: fp8 part = original cols [0, kc)
        xq[s:e, :kc] = xf[s:e, :kc].astype(ml_dtypes.float8_e4m3).astype(
            np.float32)
        xq[s:e, kc:] = xf[s:e, kc:].astype(np.float16).astype(np.float32)
    ps = xq @ q.T
    yv = ps * np.asarray(scale, np.float32)[None, :] + np.asarray(
        bias, np.float32)[None, :]
    return yv.reshape(x.shape[0], x.shape[1], -1)


def _get_program():
    key = "full"
    if key not in _PROGRAM_CACHE:
        _PROGRAM_CACHE[key] = build_program()
    return _PROGRAM_CACHE[key]


def run_on_hw(inputs, trace=False, trace_kwargs=None):
    """Run the full-size problem on 8 cores. Returns (y_full, BassKernelResults)."""
    from concourse.bass_utils import run_bass_kernel_spmd

    nc, _ = _get_program()
    in_maps = make_in_maps(inputs["x"], inputs["packed"], inputs["scale"],
                           inputs["bias"])
    kw = {}
    if trace:
        kw["trace"] = True
        if trace_kwargs:
            kw["trace_kwargs"] = trace_kwargs
    res = run_bass_kernel_spmd(nc, in_maps, core_ids=list(range(NCORES)), **kw)
    y = np.concatenate([res.results[c]["y"] for c in range(NCORES)], axis=1)
    y = np.ascontiguousarray(y.reshape(B, S, OUT), dtype=np.float32)
    return y, res


def kernel(x, packed, scale, bias):
    y, _ = run_on_hw({"x": x, "packed": packed, "scale": scale, "bias": bias})
    return y
